# revision 63
# baseline (speedup 1.0000x reference)
"""Causal GQA self-attention (RoPE + qk-RMSNorm) Trainium2 Bass kernel, v2.

Sharding over 8 NeuronCores: core = (b, g) with b = batch (2), g = kv-head
group (4). Each core computes qkv for its group's columns, RoPE + RMS norm,
causal attention for its 4 query heads against its 1 kv head, and a partial
c_proj (rows of w_proj owned by its heads). Host sums the 4 partials per
batch (row-parallel linear unshard).

v2 layout strategy (all PE-transpose-free on the hot path):
  - x arrives HOST-TRANSPOSED (C, T), so qkv is computed directly in
    (d, T) "transposed" layout: qkvT[j, t] = sum_c w[c, j] x[t, c] via
    matmul(lhsT=w_chunk, rhs=xT_chunk). No PE transposes of x.
  - RoPE rotate-half is a fixed linear map on the d axis -> one PE matmul
    with a +-1 rotation matrix R per tile; cos/sin tables are (d, T).
  - qk RMS: sum-of-squares via PE selector matmul (d on partitions),
    rsqrt = Exp(-0.5 * Ln(ms + eps)) so the scalar engine stays on the
    natural_log_exp_and_others activation table set (Exp/Ln/Square/Copy)
    and never pays a mid-kernel table switch.
  - Attention runs kb-outer (key chunk of 128 on partitions): one score
    matmul chain + ONE exp per (head, kb, tq-half) covering only the valid
    causal extent, so scalar exp work is ~minimal (causal cells + 128-col
    diagonal raggedness). Probs come out (tk, tq)-transposed, exactly what
    the v-matmul lhsT=[v|1] needs. Softmax needs no max subtraction
    (RMS-normed q,k bound |score*0.125| <= 8).
  - y head-pairs are packed on 128 partitions (odd head shifted via one
    SBUF->SBUF DMA) so c_proj runs with full 128-contraction.
  - Output partials are bf16 (halves the out DMA); host sums in fp32.
"""

import numpy as np

B, T, C = 2, 2048, 1024
NH, NKV, HD = 16, 4, 64
NREP = NH // NKV          # 4 query heads per kv group
QD = NH * HD              # 1024
KVD = NKV * HD            # 256
LQ = NREP * HD            # 256 local q cols per core
ROPE_BASE = 10000.0
EPS = 1e-6
KC = C // 128             # 8 contraction chunks for qkv
NT = T // 128             # 16 key chunks
N_CORES = 8

_CACHE: dict = {}


def _host_consts():
    import ml_dtypes
    bf16 = ml_dtypes.bfloat16
    pos = np.arange(T, dtype=np.float32)
    inv_freq = 1.0 / (ROPE_BASE ** (np.arange(0, HD, 2, dtype=np.float32) / HD))
    freqs = pos[:, None] * inv_freq[None, :]          # (T, 32)
    emb = np.repeat(freqs, 2, axis=-1)                # (T, 64) interleaved
    cos = np.cos(emb).astype(np.float32).T            # (64, T)
    sin = np.sin(emb).astype(np.float32).T
    cs2 = np.concatenate([cos, cos], axis=0).astype(bf16)   # (128, T)
    sn2 = np.concatenate([sin, sin], axis=0).astype(bf16)
    # rotation matrix: rot = R^T qk with rot[2i] = -x[2i+1], rot[2i+1] = x[2i]
    # R[p, r]: R[2i+1, 2i] = -1 ; R[2i, 2i+1] = +1, block-diag over 64-halves
    r64 = np.zeros((HD, HD), dtype=np.float32)
    for i in range(HD // 2):
        r64[2 * i + 1, 2 * i] = -1.0
        r64[2 * i, 2 * i + 1] = 1.0
    rm = np.zeros((128, 128), dtype=np.float32)
    rm[0:64, 0:64] = r64
    rm[64:128, 64:128] = r64
    rm = rm.astype(bf16)
    # sum-of-squares selectors. Stationary [128, 4]: cols map to the 4 rows
    # of one shared msq PSUM tile; jb0 writes rows 0-1, jb1 rows 2-3 via
    # zero-padded columns so both matmuls keep base partition 0.
    sel = np.zeros((128, 2), dtype=np.float32)
    sel[0:64, 0] = 1.0 / HD       # even head of chunk -> row 0
    sel[64:128, 1] = 1.0 / HD     # odd head of chunk -> row 1
    sel = sel.astype(bf16)
    # rsqrt broadcast-back selector: row 0 -> partitions 0..63,
    # row 1 -> partitions 64..127 (reused for every pair and for k)
    slb = np.zeros((2, 128), dtype=np.float32)
    slb[0, 0:64] = 1.0
    slb[1, 64:128] = 1.0
    slb = slb.astype(bf16)
    # shifted identity for v transpose (v lives on partitions 64..127)
    ids = np.zeros((128, 64), dtype=np.float32)
    for i in range(64):
        ids[64 + i, i] = 1.0
    ids = ids.astype(bf16)
    # multiplicative causal mask for the 128x128 diagonal block
    p = np.arange(128)[:, None]
    f = np.arange(128)[None, :]
    msk = (p <= f).astype(np.float32).astype(bf16)
    return cs2, sn2, rm, sel, slb, ids, msk


def _legalize_waits(nc, mybir, max_waits=1):
    """Split multi-wait instructions: this walrus build's codegen rejects
    more than one sync wait per instruction ("Too many sync wait commands"),
    so hoist all but the last wait onto standalone same-engine
    InstEventSemaphore instructions placed immediately before."""
    n_split = 0
    for func in nc.m.functions:
        for bb in func.blocks:
            need = False
            for insn in bb.instructions:
                si = insn.sync_info
                if si is not None and len(si.on_wait) > max_waits:
                    need = True
                    break
            if not need:
                continue
            out = []
            for insn in bb.instructions:
                si = insn.sync_info
                if si is not None and len(si.on_wait) > max_waits:
                    extra = list(si.on_wait)[:-max_waits]
                    keep = list(si.on_wait)[-max_waits:]
                    for j, w in enumerate(extra):
                        out.append(mybir.InstEventSemaphore(
                            name=f"{insn.name}-hw{j}",
                            engine=insn.engine,
                            sync_info=mybir.SyncInfo(on_wait=[w], on_update=[]),
                        ))
                        n_split += 1
                    insn.sync_info = mybir.SyncInfo(
                        on_wait=keep, on_update=list(si.on_update))
                out.append(insn)
            bb.instructions = out
    return n_split


def _build_nc(legalize=True, loop_iters=1):
    import concourse.bass as bass
    import concourse.tile as tile
    import concourse.mybir as mybir
    from contextlib import ExitStack, nullcontext

    f32 = mybir.dt.float32
    bf16 = mybir.dt.bfloat16
    AF = mybir.ActivationFunctionType

    nc = bass.Bass()
    xt_d = nc.dram_tensor("xt", [C, T], bf16, kind="ExternalInput")
    wq_d = nc.dram_tensor("wq", [128, KC * 384], bf16, kind="ExternalInput")
    wp_d = nc.dram_tensor("wp", [128, 2 * C], bf16, kind="ExternalInput")
    cs_d = nc.dram_tensor("cs", [128, T], bf16, kind="ExternalInput")
    sn_d = nc.dram_tensor("sn", [128, T], bf16, kind="ExternalInput")
    rm_d = nc.dram_tensor("rm", [128, 128], bf16, kind="ExternalInput")
    sel_d = nc.dram_tensor("sel", [128, 2], bf16, kind="ExternalInput")
    slb_d = nc.dram_tensor("slb", [2, 128], bf16, kind="ExternalInput")
    ids_d = nc.dram_tensor("ids", [128, 64], bf16, kind="ExternalInput")
    msk_d = nc.dram_tensor("msk", [128, 128], bf16, kind="ExternalInput")
    out_d = nc.dram_tensor("out", [T, C], bf16, kind="ExternalOutput")

    with ExitStack() as ctx:
        tc = ctx.enter_context(tile.TileContext(nc))
        const = ctx.enter_context(tc.tile_pool(name="const", bufs=1))
        resid = ctx.enter_context(tc.tile_pool(name="resid", bufs=1))

        # w first (gates the first qkv matmul), tiny consts next, the big
        # cos/sin/wp tables last (first needed ~10us in)
        w_sb = const.tile([128, KC, 384], bf16)
        for kc2 in range(4):
            nc.sync.dma_start(
                w_sb[:, 2 * kc2:2 * kc2 + 2, :],
                wq_d[:].rearrange("p (k n) -> p k n", k=KC)[
                    :, 2 * kc2:2 * kc2 + 2, :])
        rm_sb = const.tile([128, 128], bf16)
        nc.sync.dma_start(rm_sb[:], rm_d[:])
        sel_sb = const.tile([128, 2], bf16)
        nc.sync.dma_start(sel_sb[:], sel_d[:])
        slb_sb = const.tile([2, 128], bf16)
        nc.sync.dma_start(slb_sb[:], slb_d[:])
        ids_sb = const.tile([128, 64], bf16)
        nc.sync.dma_start(ids_sb[:], ids_d[:])
        msk_sb = const.tile([128, 128], bf16)
        nc.sync.dma_start(msk_sb[:], msk_d[:])
        cs_sb = const.tile([128, T], bf16)
        nc.sync.dma_start(cs_sb[:], cs_d[:])
        sn_sb = const.tile([128, T], bf16)
        nc.sync.dma_start(sn_sb[:], sn_d[:])
        wp_sb = const.tile([128, 2, C], bf16)
        nc.sync.dma_start(wp_sb[:].rearrange("p k n -> p (k n)"), wp_d[:])
        ones_sb = const.tile([128, 64], bf16)
        nc.gpsimd.memset(ones_sb[:], 1.0)
        eps_sb = const.tile([128, 1], f32)
        nc.gpsimd.memset(eps_sb[:], EPS)

        x_sb = resid.tile([128, KC, T], bf16)     # xT, c-chunk major
        qT_sb = resid.tile([128, 2, T], bf16)     # q-hat, pair-packed
        kT_sb = resid.tile([128, T], bf16)        # k-hat, dup on both halves
        va_sb = resid.tile([128, NT, HD + 1], bf16)   # [v | 1] natural
        yT_sb = resid.tile([128, 2, T], bf16)     # y pair-packed for c_proj
        yo_sb = resid.tile([64, 2, T], bf16)      # odd-head staging
        rpq_sb = resid.tile([128, 2, T], bf16)    # roped q (pre-RMS)
        rpk_sb = resid.tile([64, T], bf16)        # roped k (pre-RMS)
        msqk_sb = resid.tile([1, T], bf16)        # k mean-square staging
        msqq_sb = resid.tile([2, 2, T], bf16)     # q mean-square, [row, pair, t]
        rsk_sb = resid.tile([1, T], bf16)         # k rsqrt scale
        rsq_sb = resid.tile([2, 2, T], bf16)      # q rsqrt scales [row, pair, t]
        nc.gpsimd.memset(va_sb[:], 1.0)

        loop = tc.For_i(0, loop_iters) if loop_iters > 1 else nullcontext()
        ctx.enter_context(loop)

        # column-split x DMA so the first qkv tile can start after ~1/4 of
        # the transfer instead of waiting for the whole 4 MiB
        for tt in range(8):
            nc.sync.dma_start(
                x_sb[:, :, tt * 256:(tt + 1) * 256],
                xt_d[:].rearrange("(k p) t -> p k t", p=128)[
                    :, :, tt * 256:(tt + 1) * 256])

        # One PSUM scope, exactly 8 banks: scores 2x2, y-quarters 2x1,
        # general 2x1 (qkv/rot/msq/v-transpose/bcast/denominator/c_proj).
        with tc.tile_pool(name="ph1", bufs=5) as ph1, \
             tc.tile_pool(name="ln1", bufs=1) as ln1, \
             tc.tile_pool(name="prb", bufs=5) as prb_p, \
             tc.tile_pool(name="fin", bufs=4) as fin_p, \
             tc.tile_pool(name="osb", bufs=6) as osb_p, \
             tc.tile_pool(name="psw", bufs=2, space="PSUM") as psw, \
             tc.tile_pool(name="sml", bufs=4, space="PSUM") as sml:

            def q_rsqrt(pair):
                # windowed so each q_bcast window unblocks as soon as its
                # own Exp lands, not after the full-row chain
                for tt in range(4):
                    w0 = tt * 512
                    lnq = ln1.tile([2, 512], f32, tag="lnq",
                                   name=f"lnq{pair}{tt}")
                    nc.scalar.activation(lnq[:], msqq_sb[:, pair, w0:w0 + 512],
                                         AF.Ln, bias=eps_sb[0:2, :], scale=1.0)
                    nc.scalar.activation(rsq_sb[:, pair, w0:w0 + 512], lnq[:],
                                         AF.Exp, scale=-0.5)

            def q_bcast(pair):
                for tt in range(4):
                    w0 = tt * 512
                    ps_c = sml.tile([128, 512], f32, tag="sml",
                                    name=f"pbq{pair}{tt}")
                    nc.tensor.matmul(
                        ps_c[:], lhsT=slb_sb[0:2, :],
                        rhs=rsq_sb[0:2, pair, w0:w0 + 512],
                        start=True, stop=True)
                    nc.vector.tensor_mul(
                        qT_sb[:, pair, w0:w0 + 512],
                        rpq_sb[:, pair, w0:w0 + 512], ps_c[:])

            # ---------------- Phase 1: qkvT + rope + rms ----------------
            def k_rsqrt():
                for tt in range(4):
                    w0 = tt * 512
                    lnk = ln1.tile([1, 512], f32, tag="lnk",
                                   name=f"lnk{tt}")
                    nc.scalar.activation(lnk[:], msqk_sb[:, w0:w0 + 512],
                                         AF.Ln, bias=eps_sb[0:1, :], scale=1.0)
                    nc.scalar.activation(rsk_sb[:, w0:w0 + 512], lnk[:],
                                         AF.Exp, scale=-0.5)

            def k_bcast():
                for tt in range(4):
                    w0 = tt * 512
                    ps_c = sml.tile([128, 512], f32, tag="sml",
                                    name=f"pbk{tt}")
                    nc.tensor.matmul(
                        ps_c[0:64, :], lhsT=slb_sb[0:1, 0:64],
                        rhs=rsk_sb[0:1, w0:w0 + 512],
                        start=True, stop=True)
                    nc.vector.tensor_mul(
                        kT_sb[0:64, w0:w0 + 512],
                        rpk_sb[:, w0:w0 + 512], ps_c[0:64, :])
                    # per-window dup so odd-head scores unblock window by
                    # window instead of after the whole k row
                    nc.sync.dma_start(kT_sb[64:128, w0:w0 + 512],
                                      kT_sb[0:64, w0:w0 + 512])

            pend_act = None   # deferred Ln/Exp, emitted at next-chunk start
            pend_pe = None    # deferred bcast MMs, emitted after next chunk
            for jb in (2, 0, 1):           # kv chunk first
                nr = 64 if jb == 2 else 128    # rows that get rope+rms
                for tt in range(4):
                    w0 = tt * 512
                    if tt == 0 and pend_act is not None:
                        pend_act(); pend_act = None
                    if tt == 2 and pend_pe is not None:
                        pend_pe(); pend_pe = None
                    ps_q = psw.tile([128, 512], f32, tag="pss")
                    for kc in range(KC):
                        nc.tensor.matmul(
                            ps_q[:], lhsT=w_sb[:, kc, jb * 128:(jb + 1) * 128],
                            rhs=x_sb[:, kc, w0:w0 + 512],
                            start=(kc == 0), stop=(kc == KC - 1))
                    qk = ph1.tile([128, 512], bf16, tag="qk")
                    nc.scalar.copy(qk[:], ps_q[:])
                    if jb == 2:
                        # v rows 64..127 -> natural layout via shifted-id MM
                        for s4 in range(4):
                            pv = sml.tile([128, 64], f32, tag="sml",
                                          name=f"pv{tt}{s4}")
                            nc.tensor.matmul(
                                pv[:], lhsT=qk[64:128, s4 * 128:(s4 + 1) * 128],
                                rhs=ids_sb[64:128, :], start=True, stop=True)
                            nc.vector.tensor_copy(
                                va_sb[:, tt * 4 + s4, 0:HD], pv[:])
                    # rope: rot = R^T qk (PE), then qk*cos + rot*sin
                    ps_r = sml.tile([128, 512], f32, tag="sml")
                    nc.tensor.matmul(
                        ps_r[0:nr, :], lhsT=rm_sb[0:nr, 0:nr],
                        rhs=qk[0:nr, :], start=True, stop=True)
                    t1 = ph1.tile([128, 512], bf16, tag="t1")
                    nc.vector.tensor_mul(
                        t1[0:nr, :], qk[0:nr, :], cs_sb[0:nr, w0:w0 + 512])
                    t2 = ph1.tile([128, 512], bf16, tag="t2")
                    nc.vector.tensor_mul(
                        t2[0:nr, :], ps_r[0:nr, :], sn_sb[0:nr, w0:w0 + 512])
                    rp = (rpk_sb[:, w0:w0 + 512] if jb == 2
                          else rpq_sb[:, jb, w0:w0 + 512])
                    nc.vector.tensor_add(rp, t1[0:nr, :], t2[0:nr, :])
                    sq = ph1.tile([128, 512], bf16, tag="sq")
                    nc.vector.tensor_mul(sq[0:nr, :], rp, rp)
                    if jb == 2:
                        ps_k = sml.tile([2, 512], f32, tag="sml",
                                        name=f"msqk{tt}")
                        nc.tensor.matmul(
                            ps_k[0:1, :], lhsT=sel_sb[0:64, 0:1],
                            rhs=sq[0:64, :], start=True, stop=True)
                        nc.vector.tensor_copy(
                            msqk_sb[:, w0:w0 + 512], ps_k[0:1, :])
                    else:
                        ps_m = sml.tile([2, 512], f32, tag="sml",
                                        name=f"msqq{jb}{tt}")
                        nc.tensor.matmul(
                            ps_m[:], lhsT=sel_sb[:, 0:2],
                            rhs=sq[:, :], start=True, stop=True)
                        nc.vector.tensor_copy(
                            msqq_sb[:, jb, w0:w0 + 512], ps_m[:])
                if jb == 2:
                    pend_act, pend_pe = k_rsqrt, k_bcast
                elif jb == 0:
                    pend_act, pend_pe = (lambda: q_rsqrt(0)), (lambda: q_bcast(0))
                else:
                    q_rsqrt(1)
                    pend_pe = lambda: q_bcast(1)

            # ---------------- Phase 2 + 3: attention + c_proj ------------
            cproj_q = []
            for half in range(2):
                base = half * 1024
                for pair in range(2):
                    for h2 in range(2):
                        if pend_pe is not None and not (half == 0 and pair == 0
                                                        and h2 == 0):
                            pend_pe(); pend_pe = None
                        hp = h2 * 64
                        nkb = 8 * (half + 1)
                        ps_yq = [sml.tile([HD + 1, 512], f32, tag="sml",
                                          name=f"psy{half}{pair}{h2}{i}")
                                 for i in range(2)]
                        # pack kb chunks two-per-score-tile where extents
                        # fit in 1024 cols: one exp covers both chunks
                        kbs = []
                        for kb in range(nkb):
                            col0 = max(kb * 128, base)
                            kbs.append((kb, col0, base + 1024 - col0))
                        packs = []
                        i, j = 0, nkb - 1
                        while i <= j:
                            if i == j or kbs[i][2] + kbs[j][2] > 1024:
                                packs.append([kbs[i]]); i += 1
                            else:
                                packs.append([kbs[i], kbs[j]]); i += 1; j -= 1
                        # start/stop bookkeeping per quarter by emission order
                        touch = {0: [], 1: []}
                        for pk in packs:
                            for kb, col0, E in pk:
                                for qq in range(2):
                                    if col0 < base + qq * 512 + 512 and \
                                       col0 + E > base + qq * 512:
                                        touch[qq].append(kb)
                        for pk in packs:
                            if cproj_q:
                                cproj_q.pop(0)()
                            ps_s = psw.tile([128, 1024], f32, tag="pss")
                            off = 0
                            offs = []
                            for kb, col0, E in pk:
                                for m0 in range(0, E, 512):
                                    mw = min(512, E - m0)
                                    nc.tensor.matmul(
                                        ps_s[:, off + m0:off + m0 + mw],
                                        lhsT=kT_sb[hp:hp + 64,
                                                   kb * 128:(kb + 1) * 128],
                                        rhs=qT_sb[hp:hp + 64, pair,
                                                  col0 + m0:col0 + m0 + mw],
                                        start=True, stop=True)
                                offs.append(off)
                                off += E
                            probs = prb_p.tile([128, 1024], bf16, tag="probs")
                            nc.scalar.activation(probs[:, 0:off], ps_s[:, 0:off],
                                                 AF.Exp, scale=0.125)
                            for (kb, col0, E), po in zip(pk, offs):
                                if col0 == kb * 128:
                                    nc.gpsimd.tensor_mul(
                                        probs[:, po:po + 128],
                                        probs[:, po:po + 128], msk_sb[:])
                            for (kb, col0, E), po in zip(pk, offs):
                                for qq in range(2):
                                    s = base + qq * 512
                                    if col0 >= s + 512 or col0 + E <= s:
                                        continue
                                    lo = max(s, col0)
                                    wdt = min(s + 512, col0 + E) - lo
                                    nc.tensor.matmul(
                                        ps_yq[qq][:, lo - s:lo - s + wdt],
                                        lhsT=va_sb[:, kb, :],
                                        rhs=probs[:, po + lo - col0:
                                                  po + lo - col0 + wdt],
                                        start=(kb == touch[qq][0]),
                                        stop=(kb == touch[qq][-1]))
                        for qq in range(2):
                            s = base + qq * 512
                            rch = fin_p.tile([128, 512], bf16, tag="rch")
                            with nc.allow_low_precision(
                                    reason="1/D broadcast is bf16 anyway"):
                                nc.vector.reciprocal(
                                    rch[64:65, :], ps_yq[qq][64:65, :])
                            ps_b = sml.tile([64, 512], f32, tag="sml",
                                            name=f"psb{half}{pair}{h2}{qq}")
                            nc.tensor.matmul(
                                ps_b[:], lhsT=ones_sb[64:65, :],
                                rhs=rch[64:65, :], start=True, stop=True)
                            rcb = fin_p.tile([64, 512], bf16, tag="rcb")
                            nc.vector.tensor_copy(rcb[:], ps_b[:])
                            ydst = (yT_sb[0:64, pair, s:s + 512] if h2 == 0
                                    else yo_sb[0:64, pair, s:s + 512])
                            nc.vector.tensor_mul(
                                ydst, ps_yq[qq][0:64, :], rcb[:])
                    for qq in range(2):
                        s0 = base + qq * 512
                        nc.sync.dma_start(
                            yT_sb[64:128, pair, s0:s0 + 512],
                            yo_sb[0:64, pair, s0:s0 + 512])
                # queue this half's c_proj tiles; half-0 tiles are emitted
                # interleaved into half-1's attention stream to fill PE gaps
                def _mk_cproj(m, n2):
                    def emit():
                        ps_o = sml.tile([128, 512], f32, tag="sml",
                                        name=f"pso{m}{n2}")
                        for pair in range(2):
                            nc.tensor.matmul(
                                ps_o[:],
                                lhsT=yT_sb[:, pair, m * 128:(m + 1) * 128],
                                rhs=wp_sb[:, pair, n2 * 512:(n2 + 1) * 512],
                                start=(pair == 0), stop=(pair == 1))
                        o_sb = osb_p.tile([128, 512], bf16, tag="o_sb")
                        if (m + n2) % 2 == 0:
                            nc.vector.tensor_copy(o_sb[:], ps_o[:])
                        else:
                            nc.scalar.copy(o_sb[:], ps_o[:])
                        nc.sync.dma_start(
                            out_d[m * 128:(m + 1) * 128,
                                  n2 * 512:(n2 + 1) * 512],
                            o_sb[:])
                    return emit
                for m in range(half * 8, half * 8 + 8):
                    for n2 in range(2):
                        cproj_q.append(_mk_cproj(m, n2))
                if half == 1:
                    while cproj_q:
                        cproj_q.pop(0)()
    if legalize:
        _legalize_waits(nc, mybir)
    return nc


def _get_nc(legalize=True, loop_iters=1):
    key = ("nc", legalize, loop_iters)
    if key not in _CACHE:
        _CACHE[key] = _build_nc(legalize, loop_iters)
    return _CACHE[key]


def make_in_maps(x, w_attn, w_proj):
    import ml_dtypes
    bf16 = ml_dtypes.bfloat16
    x = np.asarray(x, dtype=np.float32)
    w_attn = np.asarray(w_attn, dtype=np.float32)
    w_proj = np.asarray(w_proj, dtype=np.float32)
    cs2, sn2, rm, sel, slb, ids, msk = _host_consts()
    in_maps = []
    xts = [np.ascontiguousarray(x[b].T).astype(bf16) for b in range(B)]
    for core in range(N_CORES):
        b, g = divmod(core, NKV)
        wq = w_attn[:, g * LQ:(g + 1) * LQ]
        wk = w_attn[:, QD + g * HD:QD + (g + 1) * HD]
        wv = w_attn[:, QD + KVD + g * HD:QD + KVD + (g + 1) * HD]
        wqkv = np.concatenate([wq, wk, wv], axis=1)          # (1024, 384)
        wq_c = np.ascontiguousarray(
            wqkv.reshape(KC, 128, 384).transpose(1, 0, 2)
        ).reshape(128, KC * 384).astype(bf16)
        # wp pair-packed: [128 (pair-head d), 2 pairs, 1024]
        wp_c = np.ascontiguousarray(
            w_proj[g * LQ:(g + 1) * LQ].reshape(2, 128, C).transpose(1, 0, 2)
        ).reshape(128, 2 * C).astype(bf16)
        in_maps.append({
            "xt": xts[b], "wq": wq_c, "wp": wp_c,
            "cs": cs2, "sn": sn2, "rm": rm, "sel": sel, "slb": slb,
            "ids": ids, "msk": msk,
        })
    return in_maps


def run_spmd(x, w_attn, w_proj, trace=False):
    from concourse.bass_utils import run_bass_kernel_spmd
    nc = _get_nc()
    in_maps = make_in_maps(x, w_attn, w_proj)
    br = run_bass_kernel_spmd(nc, in_maps, list(range(N_CORES)), trace=trace)
    out = np.empty((B, T, C), dtype=np.float32)
    for b in range(B):
        acc = br.results[NKV * b]["out"].astype(np.float32)
        for g in range(1, NKV):
            acc = acc + br.results[NKV * b + g]["out"].astype(np.float32)
        out[b] = acc
    return out, br


def kernel(x, w_attn, w_proj):
    out, _ = run_spmd(x, w_attn, w_proj, trace=False)
    return out


# revision 64
# speedup vs baseline: 1.0305x; 1.0305x over previous
"""Causal GQA self-attention (RoPE + qk-RMSNorm) Trainium2 Bass kernel, v2.

Sharding over 8 NeuronCores: core = (b, g) with b = batch (2), g = kv-head
group (4). Each core computes qkv for its group's columns, RoPE + RMS norm,
causal attention for its 4 query heads against its 1 kv head, and a partial
c_proj (rows of w_proj owned by its heads). Host sums the 4 partials per
batch (row-parallel linear unshard).

v2 layout strategy (all PE-transpose-free on the hot path):
  - x arrives HOST-TRANSPOSED (C, T), so qkv is computed directly in
    (d, T) "transposed" layout: qkvT[j, t] = sum_c w[c, j] x[t, c] via
    matmul(lhsT=w_chunk, rhs=xT_chunk). No PE transposes of x.
  - RoPE rotate-half is a fixed linear map on the d axis -> one PE matmul
    with a +-1 rotation matrix R per tile; cos/sin tables are (d, T).
  - qk RMS: sum-of-squares via PE selector matmul (d on partitions),
    rsqrt = Exp(-0.5 * Ln(ms + eps)) so the scalar engine stays on the
    natural_log_exp_and_others activation table set (Exp/Ln/Square/Copy)
    and never pays a mid-kernel table switch.
  - Attention runs kb-outer (key chunk of 128 on partitions): one score
    matmul chain + ONE exp per (head, kb, tq-half) covering only the valid
    causal extent, so scalar exp work is ~minimal (causal cells + 128-col
    diagonal raggedness). Probs come out (tk, tq)-transposed, exactly what
    the v-matmul lhsT=[v|1] needs. Softmax needs no max subtraction
    (RMS-normed q,k bound |score*0.125| <= 8).
  - y head-pairs are packed on 128 partitions (odd head shifted via one
    SBUF->SBUF DMA) so c_proj runs with full 128-contraction.
  - Output partials are bf16 (halves the out DMA); host sums in fp32.
"""

import numpy as np

B, T, C = 2, 2048, 1024
NH, NKV, HD = 16, 4, 64
NREP = NH // NKV          # 4 query heads per kv group
QD = NH * HD              # 1024
KVD = NKV * HD            # 256
LQ = NREP * HD            # 256 local q cols per core
ROPE_BASE = 10000.0
EPS = 1e-6
KC = C // 128             # 8 contraction chunks for qkv
NT = T // 128             # 16 key chunks
N_CORES = 8

_CACHE: dict = {}


def _host_consts():
    import ml_dtypes
    bf16 = ml_dtypes.bfloat16
    pos = np.arange(T, dtype=np.float32)
    inv_freq = 1.0 / (ROPE_BASE ** (np.arange(0, HD, 2, dtype=np.float32) / HD))
    freqs = pos[:, None] * inv_freq[None, :]          # (T, 32)
    emb = np.repeat(freqs, 2, axis=-1)                # (T, 64) interleaved
    cos = np.cos(emb).astype(np.float32).T            # (64, T)
    sin = np.sin(emb).astype(np.float32).T
    cs2 = np.concatenate([cos, cos], axis=0).astype(bf16)   # (128, T)
    sn2 = np.concatenate([sin, sin], axis=0).astype(bf16)
    # rotation matrix: rot = R^T qk with rot[2i] = -x[2i+1], rot[2i+1] = x[2i]
    # R[p, r]: R[2i+1, 2i] = -1 ; R[2i, 2i+1] = +1, block-diag over 64-halves
    r64 = np.zeros((HD, HD), dtype=np.float32)
    for i in range(HD // 2):
        r64[2 * i + 1, 2 * i] = -1.0
        r64[2 * i, 2 * i + 1] = 1.0
    rm = np.zeros((128, 128), dtype=np.float32)
    rm[0:64, 0:64] = r64
    rm[64:128, 64:128] = r64
    rm = rm.astype(bf16)
    # sum-of-squares selectors. Stationary [128, 4]: cols map to the 4 rows
    # of one shared msq PSUM tile; jb0 writes rows 0-1, jb1 rows 2-3 via
    # zero-padded columns so both matmuls keep base partition 0.
    sel = np.zeros((128, 2), dtype=np.float32)
    sel[0:64, 0] = 1.0 / HD       # even head of chunk -> row 0
    sel[64:128, 1] = 1.0 / HD     # odd head of chunk -> row 1
    sel = sel.astype(bf16)
    # rsqrt broadcast-back selector: row 0 -> partitions 0..63,
    # row 1 -> partitions 64..127 (reused for every pair and for k)
    slb = np.zeros((2, 128), dtype=np.float32)
    slb[0, 0:64] = 1.0
    slb[1, 64:128] = 1.0
    slb = slb.astype(bf16)
    # shifted identity for v transpose (v lives on partitions 64..127)
    ids = np.zeros((128, 64), dtype=np.float32)
    for i in range(64):
        ids[64 + i, i] = 1.0
    ids = ids.astype(bf16)
    # multiplicative causal mask for the 128x128 diagonal block
    p = np.arange(128)[:, None]
    f = np.arange(128)[None, :]
    msk = (p <= f).astype(np.float32).astype(bf16)
    return cs2, sn2, rm, sel, slb, ids, msk


def _legalize_waits(nc, mybir, max_waits=1):
    """Split multi-wait instructions: this walrus build's codegen rejects
    more than one sync wait per instruction ("Too many sync wait commands"),
    so hoist all but the last wait onto standalone same-engine
    InstEventSemaphore instructions placed immediately before."""
    n_split = 0
    for func in nc.m.functions:
        for bb in func.blocks:
            need = False
            for insn in bb.instructions:
                si = insn.sync_info
                if si is not None and len(si.on_wait) > max_waits:
                    need = True
                    break
            if not need:
                continue
            out = []
            for insn in bb.instructions:
                si = insn.sync_info
                if si is not None and len(si.on_wait) > max_waits:
                    extra = list(si.on_wait)[:-max_waits]
                    keep = list(si.on_wait)[-max_waits:]
                    for j, w in enumerate(extra):
                        out.append(mybir.InstEventSemaphore(
                            name=f"{insn.name}-hw{j}",
                            engine=insn.engine,
                            sync_info=mybir.SyncInfo(on_wait=[w], on_update=[]),
                        ))
                        n_split += 1
                    insn.sync_info = mybir.SyncInfo(
                        on_wait=keep, on_update=list(si.on_update))
                out.append(insn)
            bb.instructions = out
    return n_split


def _build_nc(legalize=True, loop_iters=1):
    import concourse.bass as bass
    import concourse.tile as tile
    import concourse.mybir as mybir
    from contextlib import ExitStack, nullcontext

    f32 = mybir.dt.float32
    bf16 = mybir.dt.bfloat16
    AF = mybir.ActivationFunctionType

    nc = bass.Bass()
    xt_d = nc.dram_tensor("xt", [C, T], bf16, kind="ExternalInput")
    wq_d = nc.dram_tensor("wq", [128, KC * 384], bf16, kind="ExternalInput")
    wp_d = nc.dram_tensor("wp", [128, 2 * C], bf16, kind="ExternalInput")
    cs_d = nc.dram_tensor("cs", [128, T], bf16, kind="ExternalInput")
    sn_d = nc.dram_tensor("sn", [128, T], bf16, kind="ExternalInput")
    rm_d = nc.dram_tensor("rm", [128, 128], bf16, kind="ExternalInput")
    sel_d = nc.dram_tensor("sel", [128, 2], bf16, kind="ExternalInput")
    slb_d = nc.dram_tensor("slb", [2, 128], bf16, kind="ExternalInput")
    ids_d = nc.dram_tensor("ids", [128, 64], bf16, kind="ExternalInput")
    msk_d = nc.dram_tensor("msk", [128, 128], bf16, kind="ExternalInput")
    out_d = nc.dram_tensor("out", [T, C], bf16, kind="ExternalOutput")

    with ExitStack() as ctx:
        tc = ctx.enter_context(tile.TileContext(nc))
        const = ctx.enter_context(tc.tile_pool(name="const", bufs=1))
        resid = ctx.enter_context(tc.tile_pool(name="resid", bufs=1))

        # w first (gates the first qkv matmul), tiny consts next, the big
        # cos/sin/wp tables last (first needed ~10us in)
        w_sb = const.tile([128, KC, 384], bf16)
        for kc2 in range(4):
            nc.sync.dma_start(
                w_sb[:, 2 * kc2:2 * kc2 + 2, :],
                wq_d[:].rearrange("p (k n) -> p k n", k=KC)[
                    :, 2 * kc2:2 * kc2 + 2, :])
        rm_sb = const.tile([128, 128], bf16)
        nc.sync.dma_start(rm_sb[:], rm_d[:])
        sel_sb = const.tile([128, 2], bf16)
        nc.sync.dma_start(sel_sb[:], sel_d[:])
        slb_sb = const.tile([2, 128], bf16)
        nc.sync.dma_start(slb_sb[:], slb_d[:])
        ids_sb = const.tile([128, 64], bf16)
        nc.sync.dma_start(ids_sb[:], ids_d[:])
        msk_sb = const.tile([128, 128], bf16)
        nc.sync.dma_start(msk_sb[:], msk_d[:])
        cs_sb = const.tile([128, T], bf16)
        nc.sync.dma_start(cs_sb[:], cs_d[:])
        sn_sb = const.tile([128, T], bf16)
        nc.sync.dma_start(sn_sb[:], sn_d[:])
        wp_sb = const.tile([128, 2, C], bf16)
        nc.sync.dma_start(wp_sb[:].rearrange("p k n -> p (k n)"), wp_d[:])
        ones_sb = const.tile([128, 64], bf16)
        nc.gpsimd.memset(ones_sb[:], 1.0)
        eps_sb = const.tile([128, 1], f32)
        nc.gpsimd.memset(eps_sb[:], EPS)

        x_sb = resid.tile([128, KC, T], bf16)     # xT, c-chunk major
        qT_sb = resid.tile([128, 2, T], bf16)     # q-hat, pair-packed
        kT_sb = resid.tile([128, T], bf16)        # k-hat, dup on both halves
        va_sb = resid.tile([128, NT, HD + 1], bf16)   # [v | 1] natural
        yT_sb = resid.tile([128, 2, T], bf16)     # y pair-packed for c_proj
        yo_sb = resid.tile([64, 2, T], bf16)      # odd-head staging
        rpq_sb = resid.tile([128, 2, T], bf16)    # roped q (pre-RMS)
        rpk_sb = resid.tile([64, T], bf16)        # roped k (pre-RMS)
        msqk_sb = resid.tile([1, T], bf16)        # k mean-square staging
        msqq_sb = resid.tile([2, 2, T], bf16)     # q mean-square, [row, pair, t]
        rsk_sb = resid.tile([1, T], bf16)         # k rsqrt scale
        rsq_sb = resid.tile([2, 2, T], bf16)      # q rsqrt scales [row, pair, t]
        nc.gpsimd.memset(va_sb[:], 1.0)

        loop = tc.For_i(0, loop_iters) if loop_iters > 1 else nullcontext()
        ctx.enter_context(loop)

        # column-split x DMA so the first qkv tile can start after ~1/4 of
        # the transfer instead of waiting for the whole 4 MiB
        for tt in range(8):
            nc.sync.dma_start(
                x_sb[:, :, tt * 256:(tt + 1) * 256],
                xt_d[:].rearrange("(k p) t -> p k t", p=128)[
                    :, :, tt * 256:(tt + 1) * 256])

        # One PSUM scope, exactly 8 banks: scores 2x2, y-quarters 2x1,
        # general 2x1 (qkv/rot/msq/v-transpose/bcast/denominator/c_proj).
        with tc.tile_pool(name="ph1", bufs=5) as ph1, \
             tc.tile_pool(name="ln1", bufs=1) as ln1, \
             tc.tile_pool(name="prb", bufs=5) as prb_p, \
             tc.tile_pool(name="fin", bufs=4) as fin_p, \
             tc.tile_pool(name="osb", bufs=6) as osb_p, \
             tc.tile_pool(name="psw", bufs=2, space="PSUM") as psw, \
             tc.tile_pool(name="sml", bufs=4, space="PSUM") as sml:

            def q_rsqrt(pair):
                # windowed so each q_bcast window unblocks as soon as its
                # own Exp lands, not after the full-row chain
                for tt in range(4):
                    w0 = tt * 512
                    lnq = ln1.tile([2, 512], f32, tag="lnq",
                                   name=f"lnq{pair}{tt}")
                    nc.scalar.activation(lnq[:], msqq_sb[:, pair, w0:w0 + 512],
                                         AF.Ln, bias=eps_sb[0:2, :], scale=1.0)
                    nc.scalar.activation(rsq_sb[:, pair, w0:w0 + 512], lnq[:],
                                         AF.Exp, scale=-0.5)

            def q_bcast(pair):
                for tt in range(4):
                    w0 = tt * 512
                    ps_c = sml.tile([128, 512], f32, tag="sml",
                                    name=f"pbq{pair}{tt}")
                    nc.tensor.matmul(
                        ps_c[:], lhsT=slb_sb[0:2, :],
                        rhs=rsq_sb[0:2, pair, w0:w0 + 512],
                        start=True, stop=True)
                    nc.vector.tensor_mul(
                        qT_sb[:, pair, w0:w0 + 512],
                        rpq_sb[:, pair, w0:w0 + 512], ps_c[:])

            # ---------------- Phase 1: qkvT + rope + rms ----------------
            def k_rsqrt():
                for tt in range(4):
                    w0 = tt * 512
                    lnk = ln1.tile([1, 512], f32, tag="lnk",
                                   name=f"lnk{tt}")
                    nc.scalar.activation(lnk[:], msqk_sb[:, w0:w0 + 512],
                                         AF.Ln, bias=eps_sb[0:1, :], scale=1.0)
                    nc.scalar.activation(rsk_sb[:, w0:w0 + 512], lnk[:],
                                         AF.Exp, scale=-0.5)

            def k_bcast():
                for tt in range(4):
                    w0 = tt * 512
                    ps_c = sml.tile([128, 512], f32, tag="sml",
                                    name=f"pbk{tt}")
                    nc.tensor.matmul(
                        ps_c[0:64, :], lhsT=slb_sb[0:1, 0:64],
                        rhs=rsk_sb[0:1, w0:w0 + 512],
                        start=True, stop=True)
                    nc.vector.tensor_mul(
                        kT_sb[0:64, w0:w0 + 512],
                        rpk_sb[:, w0:w0 + 512], ps_c[0:64, :])
                nc.sync.dma_start(kT_sb[64:128, :], kT_sb[0:64, :])

            pend_act = None   # deferred Ln/Exp, emitted at next-chunk start
            pend_pe = None    # deferred bcast MMs, emitted after next chunk
            for jb in (2, 0, 1):           # kv chunk first
                nr = 64 if jb == 2 else 128    # rows that get rope+rms
                for tt in range(4):
                    w0 = tt * 512
                    if tt == 0 and pend_act is not None:
                        pend_act(); pend_act = None
                    if tt == 2 and pend_pe is not None:
                        pend_pe(); pend_pe = None
                    ps_q = psw.tile([128, 512], f32, tag="pss")
                    for kc in range(KC):
                        nc.tensor.matmul(
                            ps_q[:], lhsT=w_sb[:, kc, jb * 128:(jb + 1) * 128],
                            rhs=x_sb[:, kc, w0:w0 + 512],
                            start=(kc == 0), stop=(kc == KC - 1))
                    qk = ph1.tile([128, 512], bf16, tag="qk")
                    nc.scalar.copy(qk[:], ps_q[:])
                    if jb == 2:
                        # v rows 64..127 -> natural layout via shifted-id MM
                        for s4 in range(4):
                            pv = sml.tile([128, 64], f32, tag="sml",
                                          name=f"pv{tt}{s4}")
                            nc.tensor.matmul(
                                pv[:], lhsT=qk[64:128, s4 * 128:(s4 + 1) * 128],
                                rhs=ids_sb[64:128, :], start=True, stop=True)
                            nc.vector.tensor_copy(
                                va_sb[:, tt * 4 + s4, 0:HD], pv[:])
                    # rope: rot = R^T qk (PE), then qk*cos + rot*sin
                    ps_r = sml.tile([128, 512], f32, tag="sml")
                    nc.tensor.matmul(
                        ps_r[0:nr, :], lhsT=rm_sb[0:nr, 0:nr],
                        rhs=qk[0:nr, :], start=True, stop=True)
                    t1 = ph1.tile([128, 512], bf16, tag="t1")
                    nc.vector.tensor_mul(
                        t1[0:nr, :], qk[0:nr, :], cs_sb[0:nr, w0:w0 + 512])
                    t2 = ph1.tile([128, 512], bf16, tag="t2")
                    nc.vector.tensor_mul(
                        t2[0:nr, :], ps_r[0:nr, :], sn_sb[0:nr, w0:w0 + 512])
                    rp = (rpk_sb[:, w0:w0 + 512] if jb == 2
                          else rpq_sb[:, jb, w0:w0 + 512])
                    nc.vector.tensor_add(rp, t1[0:nr, :], t2[0:nr, :])
                    sq = ph1.tile([128, 512], bf16, tag="sq")
                    nc.vector.tensor_mul(sq[0:nr, :], rp, rp)
                    if jb == 2:
                        ps_k = sml.tile([2, 512], f32, tag="sml",
                                        name=f"msqk{tt}")
                        nc.tensor.matmul(
                            ps_k[0:1, :], lhsT=sel_sb[0:64, 0:1],
                            rhs=sq[0:64, :], start=True, stop=True)
                        nc.vector.tensor_copy(
                            msqk_sb[:, w0:w0 + 512], ps_k[0:1, :])
                    else:
                        ps_m = sml.tile([2, 512], f32, tag="sml",
                                        name=f"msqq{jb}{tt}")
                        nc.tensor.matmul(
                            ps_m[:], lhsT=sel_sb[:, 0:2],
                            rhs=sq[:, :], start=True, stop=True)
                        nc.vector.tensor_copy(
                            msqq_sb[:, jb, w0:w0 + 512], ps_m[:])
                if jb == 2:
                    pend_act, pend_pe = k_rsqrt, k_bcast
                elif jb == 0:
                    pend_act, pend_pe = (lambda: q_rsqrt(0)), (lambda: q_bcast(0))
                else:
                    q_rsqrt(1)
                    pend_pe = lambda: q_bcast(1)

            # ---------------- Phase 2 + 3: attention + c_proj ------------
            cproj_q = []
            for half in range(2):
                base = half * 1024
                for pair in range(2):
                    for h2 in range(2):
                        if pend_pe is not None and not (half == 0 and pair == 0
                                                        and h2 == 0):
                            pend_pe(); pend_pe = None
                        hp = h2 * 64
                        nkb = 8 * (half + 1)
                        ps_yq = [sml.tile([HD + 1, 512], f32, tag="sml",
                                          name=f"psy{half}{pair}{h2}{i}")
                                 for i in range(2)]
                        # pack kb chunks two-per-score-tile where extents
                        # fit in 1024 cols: one exp covers both chunks
                        kbs = []
                        for kb in range(nkb):
                            col0 = max(kb * 128, base)
                            kbs.append((kb, col0, base + 1024 - col0))
                        packs = []
                        i, j = 0, nkb - 1
                        while i <= j:
                            if i == j or kbs[i][2] + kbs[j][2] > 1024:
                                packs.append([kbs[i]]); i += 1
                            else:
                                packs.append([kbs[i], kbs[j]]); i += 1; j -= 1
                        # start/stop bookkeeping per quarter by emission order
                        touch = {0: [], 1: []}
                        for pk in packs:
                            for kb, col0, E in pk:
                                for qq in range(2):
                                    if col0 < base + qq * 512 + 512 and \
                                       col0 + E > base + qq * 512:
                                        touch[qq].append(kb)
                        for pk in packs:
                            if cproj_q:
                                cproj_q.pop(0)()
                            ps_s = psw.tile([128, 1024], f32, tag="pss")
                            off = 0
                            offs = []
                            for kb, col0, E in pk:
                                for m0 in range(0, E, 512):
                                    mw = min(512, E - m0)
                                    nc.tensor.matmul(
                                        ps_s[:, off + m0:off + m0 + mw],
                                        lhsT=kT_sb[hp:hp + 64,
                                                   kb * 128:(kb + 1) * 128],
                                        rhs=qT_sb[hp:hp + 64, pair,
                                                  col0 + m0:col0 + m0 + mw],
                                        start=True, stop=True)
                                offs.append(off)
                                off += E
                            probs = prb_p.tile([128, 1024], bf16, tag="probs")
                            nc.scalar.activation(probs[:, 0:off], ps_s[:, 0:off],
                                                 AF.Exp, scale=0.125)
                            for (kb, col0, E), po in zip(pk, offs):
                                if col0 == kb * 128:
                                    nc.gpsimd.tensor_mul(
                                        probs[:, po:po + 128],
                                        probs[:, po:po + 128], msk_sb[:])
                            for (kb, col0, E), po in zip(pk, offs):
                                for qq in range(2):
                                    s = base + qq * 512
                                    if col0 >= s + 512 or col0 + E <= s:
                                        continue
                                    lo = max(s, col0)
                                    wdt = min(s + 512, col0 + E) - lo
                                    nc.tensor.matmul(
                                        ps_yq[qq][:, lo - s:lo - s + wdt],
                                        lhsT=va_sb[:, kb, :],
                                        rhs=probs[:, po + lo - col0:
                                                  po + lo - col0 + wdt],
                                        start=(kb == touch[qq][0]),
                                        stop=(kb == touch[qq][-1]))
                        for qq in range(2):
                            s = base + qq * 512
                            rch = fin_p.tile([128, 512], bf16, tag="rch")
                            with nc.allow_low_precision(
                                    reason="1/D broadcast is bf16 anyway"):
                                nc.vector.reciprocal(
                                    rch[64:65, :], ps_yq[qq][64:65, :])
                            ps_b = sml.tile([64, 512], f32, tag="sml",
                                            name=f"psb{half}{pair}{h2}{qq}")
                            nc.tensor.matmul(
                                ps_b[:], lhsT=ones_sb[64:65, :],
                                rhs=rch[64:65, :], start=True, stop=True)
                            rcb = fin_p.tile([64, 512], bf16, tag="rcb")
                            nc.vector.tensor_copy(rcb[:], ps_b[:])
                            ydst = (yT_sb[0:64, pair, s:s + 512] if h2 == 0
                                    else yo_sb[0:64, pair, s:s + 512])
                            nc.vector.tensor_mul(
                                ydst, ps_yq[qq][0:64, :], rcb[:])
                    for qq in range(2):
                        s0 = base + qq * 512
                        nc.sync.dma_start(
                            yT_sb[64:128, pair, s0:s0 + 512],
                            yo_sb[0:64, pair, s0:s0 + 512])
                # queue this half's c_proj tiles; half-0 tiles are emitted
                # interleaved into half-1's attention stream to fill PE gaps
                def _mk_cproj(m, n2):
                    def emit():
                        ps_o = sml.tile([128, 512], f32, tag="sml",
                                        name=f"pso{m}{n2}")
                        for pair in range(2):
                            nc.tensor.matmul(
                                ps_o[:],
                                lhsT=yT_sb[:, pair, m * 128:(m + 1) * 128],
                                rhs=wp_sb[:, pair, n2 * 512:(n2 + 1) * 512],
                                start=(pair == 0), stop=(pair == 1))
                        o_sb = osb_p.tile([128, 512], bf16, tag="o_sb")
                        if (m + n2) % 2 == 0:
                            nc.vector.tensor_copy(o_sb[:], ps_o[:])
                        else:
                            nc.scalar.copy(o_sb[:], ps_o[:])
                        nc.sync.dma_start(
                            out_d[m * 128:(m + 1) * 128,
                                  n2 * 512:(n2 + 1) * 512],
                            o_sb[:])
                    return emit
                for m in range(half * 8, half * 8 + 8):
                    for n2 in range(2):
                        cproj_q.append(_mk_cproj(m, n2))
                if half == 1:
                    while cproj_q:
                        cproj_q.pop(0)()
    if legalize:
        _legalize_waits(nc, mybir)
    return nc


def _get_nc(legalize=True, loop_iters=1):
    key = ("nc", legalize, loop_iters)
    if key not in _CACHE:
        _CACHE[key] = _build_nc(legalize, loop_iters)
    return _CACHE[key]


def make_in_maps(x, w_attn, w_proj):
    import ml_dtypes
    bf16 = ml_dtypes.bfloat16
    x = np.asarray(x, dtype=np.float32)
    w_attn = np.asarray(w_attn, dtype=np.float32)
    w_proj = np.asarray(w_proj, dtype=np.float32)
    cs2, sn2, rm, sel, slb, ids, msk = _host_consts()
    in_maps = []
    xts = [np.ascontiguousarray(x[b].T).astype(bf16) for b in range(B)]
    for core in range(N_CORES):
        b, g = divmod(core, NKV)
        wq = w_attn[:, g * LQ:(g + 1) * LQ]
        wk = w_attn[:, QD + g * HD:QD + (g + 1) * HD]
        wv = w_attn[:, QD + KVD + g * HD:QD + KVD + (g + 1) * HD]
        wqkv = np.concatenate([wq, wk, wv], axis=1)          # (1024, 384)
        wq_c = np.ascontiguousarray(
            wqkv.reshape(KC, 128, 384).transpose(1, 0, 2)
        ).reshape(128, KC * 384).astype(bf16)
        # wp pair-packed: [128 (pair-head d), 2 pairs, 1024]
        wp_c = np.ascontiguousarray(
            w_proj[g * LQ:(g + 1) * LQ].reshape(2, 128, C).transpose(1, 0, 2)
        ).reshape(128, 2 * C).astype(bf16)
        in_maps.append({
            "xt": xts[b], "wq": wq_c, "wp": wp_c,
            "cs": cs2, "sn": sn2, "rm": rm, "sel": sel, "slb": slb,
            "ids": ids, "msk": msk,
        })
    return in_maps


def run_spmd(x, w_attn, w_proj, trace=False):
    from concourse.bass_utils import run_bass_kernel_spmd
    nc = _get_nc()
    in_maps = make_in_maps(x, w_attn, w_proj)
    br = run_bass_kernel_spmd(nc, in_maps, list(range(N_CORES)), trace=trace)
    out = np.empty((B, T, C), dtype=np.float32)
    for b in range(B):
        acc = br.results[NKV * b]["out"].astype(np.float32)
        for g in range(1, NKV):
            acc = acc + br.results[NKV * b + g]["out"].astype(np.float32)
        out[b] = acc
    return out, br


def kernel(x, w_attn, w_proj):
    out, _ = run_spmd(x, w_attn, w_proj, trace=False)
    return out
